# revision 39
# baseline (speedup 1.0000x reference)
"""AMNet GNN message passing on 8 TRN2 NeuronCores.

Strategy
--------
The graph propagation ``prop(v) = scatter_add(ew * v[src]) + 0.5*v`` is an
SpMM with E=640k edges over N=10k nodes. With H=256 features, the gather
traffic of a sparse formulation (E*H*4B per hop) exceeds the cost of simply
materializing the dense normalized operator P = 0.5*I + W (W[dst,src] += ew)
once on the host and running the K=5 hops as dense GEMMs on the TensorEngine:
per hop each core multiplies its row shard P_r [1280, 10240] (fp8-e4m3,
DoubleRow) against the all-gathered node features V [10240, 256] (fp8-e4m3);
each hop's AllGather is split in two halves pipelined against the matmuls.

The Bernstein-basis + per-filter combination collapses algebraically:
  h_filters[f] = sum_i alpha[f,i] * P^i h,  alpha = sigmoid(filt_w) @ bern_coeff
so the per-hop results are folded into 5 running accumulators HF_f on the fly.
Attention fusion (tanh projections, logits, softmax, res) runs sharded on the
node dimension, and the final res @ res.T is a distributed GEMM over
row-sharded res with an all-gathered (transposed) copy of res.

Nodes are padded 10000 -> 10240 so every core owns exactly 1280 = 10x128 rows
and all 8 cores run an identical SPMD graph; padding rows are all-zero and are
sliced away on the host after gathering the 8 output slabs.
"""

import sys

for _p in ("/opt/trn_rl_repo", "/root/.axon_site/_ro/trn_rl_repo"):
    if _p not in sys.path:
        sys.path.append(_p)

import math

import ml_dtypes
import numpy as np

import concourse.bass as bass
import concourse.mybir as mybir
import concourse.tile as tile
from concourse import bacc
from concourse.bass_utils import run_bass_kernel_spmd
from concourse.masks import make_identity

N, E, IN, HID, KDEG, FN = 10000, 640000, 512, 256, 5, 5
NCORES = 8
NPAD = 10240                 # padded node count (8 * 1280)
SH = NPAD // NCORES          # 1280 nodes per core
MT = SH // 128               # 10 m-tiles per core
KT = NPAD // 128             # 80 k-tiles (contraction over all nodes)
HT = HID // 128              # 2 feature tiles
INT = IN // 128              # 4 input-feature tiles

BF16 = mybir.dt.bfloat16
F32 = mybir.dt.float32
HOP_FP8 = True
FP8 = mybir.dt.float8e4
HOP_DT = FP8 if HOP_FP8 else BF16


def _bern_coeff(K):
    out = np.zeros((K + 1, K + 1), dtype=np.float64)
    P = np.polynomial.polynomial.Polynomial
    for i in range(K + 1):
        c = np.zeros(i + 1)
        c[i] = math.comb(K, i)
        p = P(c) * (P([1.0, -1.0]) ** (K - i))
        out[i, : len(p.coef)] = p.coef
    return out


def _build_graph():
    """Build the SPMD Bass graph (identical on all 8 cores)."""
    nc = bacc.Bacc("TRN2", target_bir_lowering=False, debug=False, num_devices=NCORES)

    # ---- per-core parameters ----
    xt = nc.declare_dram_parameter("xt", [IN, SH], BF16, isOutput=False)
    ptt = nc.declare_dram_parameter("ptt", [MT, 128, KT, 128], HOP_DT, isOutput=False)
    w1 = nc.declare_dram_parameter("w1", [IN, HID], BF16, isOutput=False)
    w2 = nc.declare_dram_parameter("w2", [HID, HID], BF16, isOutput=False)
    wf = nc.declare_dram_parameter("wf", [HID, HID], BF16, isOutput=False)
    wx = nc.declare_dram_parameter("wx", [HID, HID], BF16, isOutput=False)
    b1 = nc.declare_dram_parameter("b1", [128, HT], F32, isOutput=False)
    b2 = nc.declare_dram_parameter("b2", [128, HT], F32, isOutput=False)
    wfb = nc.declare_dram_parameter("wfb", [128, HID], BF16, isOutput=False)
    wxb = nc.declare_dram_parameter("wxb", [128, HID], BF16, isOutput=False)
    onesr = nc.declare_dram_parameter("onesr", [128, 128], BF16, isOutput=False)
    alpha = nc.declare_dram_parameter("alpha", [128, FN * (KDEG + 1)], F32, isOutput=False)
    out = nc.declare_dram_parameter("out", [SH, NPAD], F32, isOutput=True)

    rg = [list(range(NCORES))]

    with tile.TileContext(nc) as tc:
        with tc.tile_pool(name="dram", bufs=1, space="DRAM") as dram, \
             tc.tile_pool(name="consts", bufs=1) as consts, \
             tc.tile_pool(name="persist", bufs=1) as persist:

            # ---- DRAM internals for collectives ----
            # All AllGather payloads are partition-major ([128, tiles, feat]) so
            # every DMA touching them is contiguous per partition. Rank r's
            # block of the AG output holds nodes r*1280 + t*128 + p, which is
            # exactly k-tile kt = r*10 + t, partition p of the node dimension.
            # hop V exchange: h uses one full AllGather; hops 1..4 split the
            # AllGather into 3 segments (t 0..5 after m5, t 6..8 after m8,
            # t 9 after m9) so only the tiny last segment's latency is exposed
            SEG = [(0, 6), (6, 9), (9, 10)]
            bounce_h = dram.tile([128, MT, HID], HOP_DT)
            vfull_h = dram.tile([NCORES, 128, MT, HID], HOP_DT,
                                addr_space="Shared")
            bounce_s = [[
                dram.tile([128, hi - lo, HID], HOP_DT, name=f"bounce_s{i}_{j}")
                for j, (lo, hi) in enumerate(SEG)] for i in range(KDEG)
            ]
            vfull_s = [[
                dram.tile([NCORES, 128, hi - lo, HID], HOP_DT,
                          addr_space="Shared", name=f"vfull_s{i}_{j}")
                for j, (lo, hi) in enumerate(SEG)] for i in range(KDEG)
            ]
            RA, RB = 512, SH - 512        # res.T column split (rank-aligned)
            res_bounce_a = dram.tile([128, HT, RA], BF16)
            res_bounce_b = dram.tile([128, HT, RB], BF16)
            res_ag_a = dram.tile([NCORES, 128, HT, RA], BF16, addr_space="Shared")
            res_ag_b = dram.tile([NCORES, 128, HT, RB], BF16, addr_space="Shared")

            # warm up the collectives path: tiny AllGather with no deps so
            # the first-real AG doesn't pay the ncfw cold cost serially
            warm_in = dram.tile([128, 2], F32)
            warm_out = dram.tile([NCORES, 128, 2], F32, addr_space="Shared")
            nc.gpsimd.collective_compute(
                "AllGather", mybir.AluOpType.bypass,
                ins=[warm_in.opt()], outs=[warm_out.opt()], replica_groups=rg,
            )

            # ---- constants to SBUF ----
            w1_sb = consts.tile([128, INT, HID], BF16)
            nc.sync.dma_start(w1_sb[:], w1[:, :].rearrange("(kt p) n -> p kt n", p=128))
            w2_sb = consts.tile([128, HT, HID], BF16)
            nc.sync.dma_start(w2_sb[:], w2[:, :].rearrange("(kt p) n -> p kt n", p=128))
            wf_sb = consts.tile([128, HT, HID], BF16)
            nc.sync.dma_start(wf_sb[:], wf[:, :].rearrange("(kt p) n -> p kt n", p=128))
            wx_sb = consts.tile([128, HT, HID], BF16)
            nc.sync.dma_start(wx_sb[:], wx[:, :].rearrange("(kt p) n -> p kt n", p=128))
            b1_sb = consts.tile([128, HT], F32)
            nc.sync.dma_start(b1_sb[:], b1[:, :])
            b2_sb = consts.tile([128, HT], F32)
            nc.sync.dma_start(b2_sb[:], b2[:, :])
            wfb_sb = consts.tile([128, HID], BF16)
            nc.sync.dma_start(wfb_sb[:], wfb[:, :])
            wxb_sb = consts.tile([128, HID], BF16)
            nc.sync.dma_start(wxb_sb[:], wxb[:, :])
            ones_sb = consts.tile([128, 128], BF16)
            nc.sync.dma_start(ones_sb[:], onesr[:, :])
            alpha_sb = consts.tile([128, FN * (KDEG + 1)], F32)
            nc.sync.dma_start(alpha_sb[:], alpha[:, :])
            ident = consts.tile([128, 128], BF16)
            make_identity(nc, ident[:])
            ident32 = consts.tile([128, 128], F32)
            make_identity(nc, ident32[:])

            # ---- persistent activations ----
            resT_sb = persist.tile([128, HT, SH], BF16)     # res.T (for final GEMM)

            NCH = [(0, 512), (512, 512), (1024, 256)]       # 1280 split into <=512

            hf_ctx = tc.tile_pool(name="hf", bufs=1)
            hf_pool = hf_ctx.__enter__()
            hf_sb = hf_pool.tile([128, FN, MT, HID], F32)   # HF_f (node-major)

            xp_sb = persist.tile([128, MT, HID], BF16)      # x_proj (node-major)

            with tc.tile_pool(name="tr_ps", bufs=2, space="PSUM") as tr_ps, \
                 tc.tile_pool(name="hop_sb", bufs=1) as hop_sb, \
                 tc.tile_pool(name="pt_sb", bufs=4) as pt_sb, \
                 tc.tile_pool(name="vsb", bufs=2) as vsb_pool, \
                 tc.tile_pool(name="hop_ps", bufs=4, space="PSUM") as hop_ps, \
                 tc.tile_pool(name="tmp", bufs=3) as tmp_pool:

                # ======== MLP: h = relu(x@W1+b1)@W2+b2, computed as h.T ========
                hT_sb = hop_sb.tile([128, HT, SH], BF16, name="hT_sb")
                with tc.tile_pool(name="mlp", bufs=1) as mlp, \
                     tc.tile_pool(name="mlp_ps", bufs=2, space="PSUM") as mlp_ps:
                    xt_sb = mlp.tile([128, INT, SH], BF16)
                    xt_r = xt[:, :].rearrange("(kt p) n -> p kt n", p=128)
                    for kt in range(INT):
                        for c in range(3):
                            nc.sync.dma_start(
                                xt_sb[:, kt, c * 512:min((c + 1) * 512, SH)],
                                xt_r[:, kt, c * 512:min((c + 1) * 512, SH)],
                            )

                    h1_sb = mlp.tile([128, HT, SH], BF16)
                    for mo in range(HT):
                        for n0, nw in NCH:
                            ps_t = mlp_ps.tile([128, 512], F32, name="mlp_psum")
                            for k in range(INT):
                                nc.tensor.matmul(
                                    ps_t[:, :nw],
                                    w1_sb[:, k, mo * 128:(mo + 1) * 128],
                                    xt_sb[:, k, n0:n0 + nw],
                                    start=(k == 0), stop=(k == INT - 1),
                                )
                            nc.scalar.activation(
                                h1_sb[:, mo, n0:n0 + nw], ps_t[:, :nw],
                                mybir.ActivationFunctionType.Relu,
                                bias=b1_sb[:, mo:mo + 1],
                            )
                    for mo in range(HT):
                        for n0, nw in NCH:
                            ps_t = mlp_ps.tile([128, 512], F32, name="mlp_psum2", tag="mlp_psum")
                            for k in range(HT):
                                nc.tensor.matmul(
                                    ps_t[:, :nw],
                                    w2_sb[:, k, mo * 128:(mo + 1) * 128],
                                    h1_sb[:, k, n0:n0 + nw],
                                    start=(k == 0), stop=(k == HT - 1),
                                )
                            nc.scalar.activation(
                                hT_sb[:, mo, n0:n0 + nw], ps_t[:, :nw],
                                mybir.ActivationFunctionType.Identity,
                                bias=b2_sb[:, mo:mo + 1],
                            )

                # h node-major + HF init (i=0) + bounce for AllGather (halves)
                v_loc = hop_sb.tile([128, MT, HID], BF16, name="v_loc")
                v_loc8 = hop_sb.tile([128, MT, HID], HOP_DT, name="v_loc8")
                for t in range(MT):
                    for mo in range(HT):
                        ps_tr = tr_ps.tile([128, 128], BF16, name="tr_psum")
                        nc.tensor.transpose(
                            ps_tr[:], hT_sb[:, mo, t * 128:(t + 1) * 128], ident[:]
                        )
                        nc.vector.tensor_copy(
                            v_loc[:, t, mo * 128:(mo + 1) * 128], ps_tr[:]
                        )
                    nc.vector.tensor_copy(v_loc8[:, t, :], v_loc[:, t, :])
                    for f in range(FN):
                        nc.scalar.activation(
                            hf_sb[:, f, t, :], v_loc[:, t, :],
                            mybir.ActivationFunctionType.Copy,
                            scale=alpha_sb[:, f * (KDEG + 1):f * (KDEG + 1) + 1],
                        )
                nc.sync.dma_start(bounce_h[:, :, :], v_loc8[:])
                nc.gpsimd.collective_compute(
                    "AllGather", mybir.AluOpType.bypass,
                    ins=[bounce_h.opt()], outs=[vfull_h.opt()], replica_groups=rg,
                )

                # x_proj = tanh(h @ wx + wxb) — emitted after the AG trigger so
                # the PE chews on it while the first AllGather is in flight
                for m in range(MT):
                    ps_x = hop_ps.tile([128, HID], F32, name="xp_psum", tag="hop_psum")
                    for ko in range(HT):
                        nc.tensor.matmul(
                            ps_x[:],
                            hT_sb[:, ko, m * 128:(m + 1) * 128],
                            wx_sb[:, ko, :],
                            start=(ko == 0), stop=False,
                        )
                    nc.tensor.matmul(
                        ps_x[:], ones_sb[:], wxb_sb[:], start=False, stop=True
                    )
                    nc.scalar.activation(
                        xp_sb[:, m, :], ps_x[:], mybir.ActivationFunctionType.Tanh
                    )

                # ======== 5 propagation hops ========
                NLEAD = 4

                def reload_seg(v_dst, src_list, lo, hi):
                    for r in range(NCORES):
                        nc.sync.dma_start(
                            v_dst[:, r * MT + lo:r * MT + hi, :], src_list[r]
                        )

                def prefetch_pts(hop, pts_d, psums_d):
                    for m in range(NLEAD):
                        pt_t = pt_sb.tile([128, KT, 128], HOP_DT, tag="pt",
                                          name=f"pt{hop}_{m}")
                        nc.sync.dma_start(pt_t[:], ptt[m])
                        pts_d[m] = pt_t
                        psums_d[m] = hop_ps.tile([128, HID], F32,
                                                 name=f"hop_psum{hop}_{m}",
                                                 tag="hop_psum")

                v_sb_cur = vsb_pool.tile([128, KT, HID], HOP_DT, tag="v_sb",
                                         name="v_sb1")
                pts_cur, psums_cur = {}, {}
                prefetch_pts(1, pts_cur, psums_cur)
                reload_seg(v_sb_cur, [vfull_h[r] for r in range(NCORES)], 0, MT)
                for hop in range(1, KDEG + 1):
                    v_sb = v_sb_cur
                    v_next = hop_sb.tile([128, MT, HID], HOP_DT, name=f"v_next{hop}",
                                         tag="v_next")
                    # k-pair order follows segment availability: pairs fully
                    # inside seg0 first, then (6,7), then (8,9)
                    ks1 = [r * MT + t for r in range(NCORES) for t in (0, 2, 4)]
                    ks2 = [r * MT + 6 for r in range(NCORES)]
                    ks3 = [r * MT + 8 for r in range(NCORES)]
                    korder = ks1 + ks2 + ks3
                    steps = [(m, "F") for m in range(MT)]
                    pts, psums = pts_cur, psums_cur

                    def mm_group(m, klist, first, last):
                        if m not in pts:
                            pt_t = pt_sb.tile([128, KT, 128], HOP_DT, tag="pt",
                                              name=f"pt{hop}_{m}")
                            nc.sync.dma_start(pt_t[:], ptt[m])
                            pts[m] = pt_t
                            psums[m] = hop_ps.tile([128, HID], F32,
                                                   name=f"hop_psum{hop}_{m}",
                                                   tag="hop_psum")
                        pt_t, ps_m = pts[m], psums[m]
                        if HOP_FP8:
                            for i, k in enumerate(klist):
                                nc.tensor.matmul(
                                    ps_m[:], pt_t[:, k:k + 2, :], v_sb[:, k:k + 2, :],
                                    start=(first and i == 0),
                                    stop=(last and i == len(klist) - 1),
                                    perf_mode=mybir.MatmulPerfMode.DoubleRow,
                                )
                        else:
                            for i, k in enumerate(klist):
                                for k2 in (k, k + 1):
                                    nc.tensor.matmul(
                                        ps_m[:], pt_t[:, k2, :], v_sb[:, k2, :],
                                        start=(first and i == 0 and k2 == k),
                                        stop=(last and i == len(klist) - 1
                                              and k2 == k + 1),
                                    )

                    for m, ph in steps:
                        mm_group(m, korder, True, True)
                        ps_m = psums[m]
                        if hop < KDEG:
                            nc.vector.tensor_copy(v_next[:, m, :], ps_m[:])
                            for j, (lo, hi) in enumerate(SEG[:-1]):
                                if m == hi - 1:
                                    nc.sync.dma_start(
                                        bounce_s[hop][j][:, :, :],
                                        v_next[:, lo:hi, :],
                                    )
                                    nc.gpsimd.collective_compute(
                                        "AllGather", mybir.AluOpType.bypass,
                                        ins=[bounce_s[hop][j].opt()],
                                        outs=[vfull_s[hop][j].opt()],
                                        replica_groups=rg,
                                    )
                        for f in range(FN):
                            sc = tmp_pool.tile([128, HID], F32, tag="hf_tmp")
                            nc.scalar.activation(
                                sc[:], ps_m[:],
                                mybir.ActivationFunctionType.Copy,
                                scale=alpha_sb[:, f * (KDEG + 1) + hop:
                                               f * (KDEG + 1) + hop + 1],
                            )
                            nc.vector.tensor_add(
                                out=hf_sb[:, f, m, :], in0=hf_sb[:, f, m, :], in1=sc[:]
                            )
                    if hop < KDEG:
                        v_sb_cur = vsb_pool.tile([128, KT, HID], HOP_DT,
                                                 tag="v_sb", name=f"v_sb{hop + 1}")
                        pts_cur, psums_cur = {}, {}
                        lo0, hi0 = SEG[0]
                        reload_seg(v_sb_cur,
                                   [vfull_s[hop][0][r] for r in range(NCORES)],
                                   lo0, hi0)
                        lo2, hi2 = SEG[2]
                        nc.sync.dma_start(
                            bounce_s[hop][2][:, :, :], v_next[:, lo2:hi2, :]
                        )
                        nc.gpsimd.collective_compute(
                            "AllGather", mybir.AluOpType.bypass,
                            ins=[bounce_s[hop][2].opt()],
                            outs=[vfull_s[hop][2].opt()], replica_groups=rg,
                        )
                        prefetch_pts(hop + 1, pts_cur, psums_cur)
                        lo1, hi1 = SEG[1]
                        reload_seg(v_sb_cur,
                                   [vfull_s[hop][1][r] for r in range(NCORES)],
                                   lo1, hi1)
                        reload_seg(v_sb_cur,
                                   [vfull_s[hop][2][r] for r in range(NCORES)],
                                   lo2, hi2)

            # ======== attention fusion (node-sharded) ========
            with tc.tile_pool(name="attn", bufs=1) as attn, \
                 tc.tile_pool(name="attn_ps", bufs=2, space="PSUM") as attn_ps, \
                 tc.tile_pool(name="tr_ps2", bufs=2, space="PSUM") as tr_ps2, \
                 tc.tile_pool(name="tmp2", bufs=3) as tmp2:

                # HF_f.T via PE transposes (feature-major, for the wf matmul)
                hft_sb = attn.tile([128, FN, HT, SH], BF16)
                for f in range(FN):
                    for mo in range(HT):
                        for t in range(MT):
                            ps_tr = tr_ps2.tile([128, 128], F32, name="tr2_psum")
                            nc.tensor.transpose(
                                ps_tr[:],
                                hf_sb[:, f, t, mo * 128:(mo + 1) * 128],
                                ident32[:],
                            )
                            nc.vector.tensor_copy(
                                hft_sb[:, f, mo, t * 128:(t + 1) * 128], ps_tr[:]
                            )

                # hf_proj = tanh(HF_f @ wf + wfb)   (node-major)
                hfp_sb = attn.tile([128, FN, MT, HID], BF16)
                for f in range(FN):
                    for m in range(MT):
                        ps_p = attn_ps.tile([128, HID], F32, name="attn_psum")
                        for ko in range(HT):
                            nc.tensor.matmul(
                                ps_p[:],
                                hft_sb[:, f, ko, m * 128:(m + 1) * 128],
                                wf_sb[:, ko, :],
                                start=(ko == 0), stop=False,
                            )
                        nc.tensor.matmul(
                            ps_p[:], ones_sb[:], wfb_sb[:], start=False, stop=True
                        )
                        nc.scalar.activation(
                            hfp_sb[:, f, m, :], ps_p[:],
                            mybir.ActivationFunctionType.Tanh,
                        )

                # logits, softmax over the FN filters, res (all node-major)
                score_sb = attn.tile([128, MT, FN], F32)
                for m in range(MT):
                    logit = tmp2.tile([128, FN], F32, tag="logit")
                    for f in range(FN):
                        prod = tmp2.tile([128, HID], F32, tag="prod")
                        nc.vector.affine_mul_reduce(
                            prod[:], logit[:, f:f + 1],
                            hfp_sb[:, f, m, :], xp_sb[:, m, :], 1.0, 0.0,
                        )
                    mx = tmp2.tile([128, 1], F32, tag="mx")
                    nc.vector.tensor_reduce(
                        mx[:], logit[:], axis=mybir.AxisListType.X,
                        op=mybir.AluOpType.max,
                    )
                    es = tmp2.tile([128, FN], F32, tag="es")
                    nc.vector.tensor_scalar(
                        es[:], logit[:], mx[:], None, mybir.AluOpType.subtract
                    )
                    nc.scalar.activation(
                        es[:], es[:], mybir.ActivationFunctionType.Exp
                    )
                    sm = tmp2.tile([128, 1], F32, tag="sm")
                    nc.vector.tensor_reduce(
                        sm[:], es[:], axis=mybir.AxisListType.X, op=mybir.AluOpType.add
                    )
                    rinv = tmp2.tile([128, 1], F32, tag="rinv")
                    nc.vector.reciprocal(rinv[:], sm[:])
                    nc.vector.tensor_scalar(
                        score_sb[:, m, :], es[:], rinv[:], None, mybir.AluOpType.mult
                    )

                # res = sum_f score_f * HF_f  -> transpose -> res.T -> AllGather
                for m in range(MT):
                    racc = tmp2.tile([128, HID], F32, tag="racc")
                    nc.scalar.activation(
                        racc[:], hf_sb[:, 0, m, :],
                        mybir.ActivationFunctionType.Copy,
                        scale=score_sb[:, m, 0:1],
                    )
                    for f in range(1, FN):
                        sc = tmp2.tile([128, HID], F32, tag="rsc")
                        nc.scalar.activation(
                            sc[:], hf_sb[:, f, m, :],
                            mybir.ActivationFunctionType.Copy,
                            scale=score_sb[:, m, f:f + 1],
                        )
                        nc.vector.tensor_add(out=racc[:], in0=racc[:], in1=sc[:])
                    res_bf = tmp2.tile([128, HID], BF16, tag="res_bf")
                    nc.vector.tensor_copy(res_bf[:], racc[:])
                    for ko in range(HT):
                        ps_tr = tr_ps2.tile([128, 128], BF16, name="tr3_psum")
                        nc.tensor.transpose(
                            ps_tr[:], res_bf[:, ko * 128:(ko + 1) * 128], ident[:]
                        )
                        nc.vector.tensor_copy(
                            resT_sb[:, ko, m * 128:(m + 1) * 128], ps_tr[:]
                        )
                    if m == RA // 128 - 1:
                        nc.sync.dma_start(
                            res_bounce_a[:, :, :], resT_sb[:, :, :RA]
                        )
                        nc.gpsimd.collective_compute(
                            "AllGather", mybir.AluOpType.bypass,
                            ins=[res_bounce_a.opt()], outs=[res_ag_a.opt()],
                            replica_groups=rg,
                        )
                nc.sync.dma_start(res_bounce_b[:, :, :], resT_sb[:, :, RA:])
                nc.gpsimd.collective_compute(
                    "AllGather", mybir.AluOpType.bypass,
                    ins=[res_bounce_b.opt()], outs=[res_ag_b.opt()],
                    replica_groups=rg,
                )
            hf_ctx.__exit__(None, None, None)

            # ======== final distributed GEMM: out_r = res_r @ res_full.T ========
            with tc.tile_pool(name="fin", bufs=1) as fin, \
                 tc.tile_pool(name="stage", bufs=3) as stage_pool, \
                 tc.tile_pool(name="fin_ps", bufs=6, space="PSUM") as fin_ps:
                rhs_sb = fin.tile([128, HT, NPAD], BF16)
                for ko in range(HT):
                    for r in range(NCORES):
                        nc.sync.dma_start(
                            rhs_sb[:, ko, r * SH:r * SH + RA],
                            res_ag_a[r, :, ko, :],
                        )
                        nc.sync.dma_start(
                            rhs_sb[:, ko, r * SH + RA:(r + 1) * SH],
                            res_ag_b[r, :, ko, :],
                        )
                out_r = out[:, :].rearrange("(t p) f -> p t f", p=128)
                # chunks aligned to the rank blocks: per rank an A chunk
                # (512) then B chunks (512 + 256); A chunks of every rank run
                # first so the GEMM starts while the B-half is still gathering
                CHUNKS_A = [(r * SH, 512) for r in range(NCORES)]
                CHUNKS_B = [(r * SH + 512, 512) for r in range(NCORES)] + \
                           [(r * SH + 1024, 256) for r in range(NCORES)]
                for m in range(MT):
                    stage = stage_pool.tile([128, NPAD], F32, tag="stage")
                    for half, chunks in (("a", CHUNKS_A), ("b", CHUNKS_B)):
                        for c0, cw in chunks:
                            ps_f = fin_ps.tile([128, 512], F32, name="fin_psum")
                            for ko in range(HT):
                                nc.tensor.matmul(
                                    ps_f[:, :cw],
                                    resT_sb[:, ko, m * 128:(m + 1) * 128],
                                    rhs_sb[:, ko, c0:c0 + cw],
                                    start=(ko == 0), stop=(ko == HT - 1),
                                )
                            nc.vector.tensor_copy(
                                stage[:, c0:c0 + cw], ps_f[:, :cw]
                            )
                        stage_r = stage.rearrange("p (r q) -> p r q", q=SH)
                        dst_r = out_r[:, m, :].rearrange("p (r q) -> p r q", q=SH)
                        if half == "a":
                            nc.sync.dma_start(
                                dst_r[:, :, :RA], stage_r[:, :, :RA]
                            )
                        else:
                            nc.sync.dma_start(
                                dst_r[:, :, RA:], stage_r[:, :, RA:]
                            )
    nc.compile()
    return nc


_GRAPH_CACHE = {}


def _get_graph():
    if "nc" not in _GRAPH_CACHE:
        _GRAPH_CACHE["nc"] = _build_graph()
    return _GRAPH_CACHE["nc"]


def prepare_in_maps(x, edge_index, lin1_w, lin1_b, lin2_w, lin2_b, filt_w,
                    wf_w, wf_b, wx_w, wx_b):
    x = np.asarray(x, np.float32)
    edge_index = np.asarray(edge_index)
    src = edge_index[0].astype(np.int64)
    dst = edge_index[1].astype(np.int64)

    # ---- host prep: dense normalized propagation operator ----
    deg = np.zeros(N, np.float32)
    np.add.at(deg, src, np.float32(1.0))
    dinv = np.where(deg > 0, 1.0 / np.sqrt(deg), 0.0).astype(np.float32)
    ew = (-(dinv[src] * dinv[dst]) * 0.5).astype(np.float32)
    P = np.zeros((NPAD, NPAD), np.float32)
    np.add.at(P, (dst, src), ew)
    P[np.arange(NPAD), np.arange(NPAD)] += 0.5

    coeff = _bern_coeff(KDEG).astype(np.float32)
    fw = (1.0 / (1.0 + np.exp(-np.asarray(filt_w, np.float32))))
    alpha = (fw @ coeff).astype(np.float32)             # [FN, KDEG+1]
    alpha_bc = np.repeat(alpha.reshape(1, -1), 128, 0).astype(np.float32)

    xpad = np.zeros((NPAD, IN), np.float32)
    xpad[:N] = x

    bf = ml_dtypes.bfloat16
    hop_np = ml_dtypes.float8_e4m3 if HOP_FP8 else bf
    w1_b = np.ascontiguousarray(np.asarray(lin1_w, np.float32)).astype(bf)
    w2_b = np.ascontiguousarray(np.asarray(lin2_w, np.float32)).astype(bf)
    wf_bm = np.ascontiguousarray(np.asarray(wf_w, np.float32)).astype(bf)
    wx_bm = np.ascontiguousarray(np.asarray(wx_w, np.float32)).astype(bf)
    b1_bc = np.zeros((128, HT), np.float32)
    b1_bc[:] = np.asarray(lin1_b, np.float32).reshape(HT, 128).T
    b2_bc = np.zeros((128, HT), np.float32)
    b2_bc[:] = np.asarray(lin2_b, np.float32).reshape(HT, 128).T
    wfb_row = np.zeros((128, HID), np.float32)
    wfb_row[0] = np.asarray(wf_b, np.float32)
    wxb_row = np.zeros((128, HID), np.float32)
    wxb_row[0] = np.asarray(wx_b, np.float32)
    ones_row = np.zeros((128, 128), np.float32)
    ones_row[0] = 1.0

    in_maps = []
    for r in range(NCORES):
        rows = slice(r * SH, (r + 1) * SH)
        # lhsT layout: ptt[m_t, p, k_t, f] = P[r*SH + m_t*128 + f, k_t*128 + p]
        A = P[rows].astype(hop_np)                            # [SH, NPAD]
        ptt = np.ascontiguousarray(
            A.reshape(MT, 128, KT, 128).transpose(0, 3, 2, 1)
        )
        xt = np.ascontiguousarray(xpad[rows].T).astype(bf)    # [IN, SH]
        in_maps.append(dict(
            xt=xt, ptt=ptt, w1=w1_b, w2=w2_b, wf=wf_bm, wx=wx_bm,
            b1=b1_bc, b2=b2_bc,
            wfb=wfb_row.astype(bf), wxb=wxb_row.astype(bf),
            onesr=ones_row.astype(bf), alpha=alpha_bc,
        ))
    return in_maps


def run(in_maps, trace=False, **kw):
    nc = _get_graph()
    return run_bass_kernel_spmd(
        nc, in_maps, core_ids=list(range(NCORES)), trace=trace, **kw
    )


def kernel(**inputs):
    in_maps = prepare_in_maps(**inputs)
    res = run(in_maps)
    full = np.concatenate([res.results[r]["out"] for r in range(NCORES)], 0)
    return np.ascontiguousarray(full[:N, :N]).astype(np.float32)
